# revision 50
# baseline (speedup 1.0000x reference)
"""Trainium2 Bass kernel for nn_Attention_D (pairwise-bias attention).

Problem: B=2, N=256, C=768, H=12, hd=64
  qkv = x @ w_qkv.T ; attn = softmax(q k^T * hd^-0.5)
  out = attn @ v + einsum('bhij,bhijd->bhid', attn, dh); out @ w_proj.T + b

d [B, N, N, C] dominates; the kernel is DMA-bound (global DMA pipe
~360 B/ns per core). Query rows are sharded across the 8 cores (32 per
batch per core); d streams in as float8_e3m4 (1 byte, ~1e-2 rel err vs
the 2e-2 gate; fp8 weights/attn tested and rejected), everything else
fp16. Per-core DMA ~51.4 us of the ~58.7 us total.

The d-term out2[h,i,c] = sum_j attn[h,i,j] * d[i,j,c] is computed entirely
on PE in transposed form: per token i, per 128-wide c-chunk ck and 64-wide
half (head h = 2*ck + half), a tiny matmul
    psum[c, i-col] += d_i[j, c-block].T(lhsT, e3m4) @ attnT[:, h-col](fp16)
accumulates the diagonal-block result directly into a [c, token] PSUM
layout (out free size 1 -> ~no PE time; PE reads e3m4 straight from the
DMA tile; mixed e3m4 x fp16 matmul validated on HW). The v-term
(v.T as lhsT, attnT as rhs) accumulates into the same PSUM region, so the
epilogue is a single PSUM->SBUF fp16 copy per (b, ck) producing hfinT in
exactly the lhsT layout the final projection needs. No DVE d-path, no
diagonal extraction, no transposes of the output.
"""

import numpy as np
import ml_dtypes

import concourse.bass as bass
import concourse.bacc as bacc
import concourse.mybir as mybir
import concourse.tile as tile
from concourse.bass_utils import run_bass_kernel_spmd

B, N, C = 2, 256, 768
H, HD = 12, 64
NCORES = 8
TOK = N // NCORES          # 32 own query rows per batch per core
CK = C // 128              # 6 c-chunks
JT = N // 128              # 2 j partition tiles
NTOK = 4                   # tokens per d DMA chunk
TOKA = 24                  # batch-1 tokens projected early (rest: tail path)
TOKB = TOK - TOKA
F32 = mybir.dt.float32
F16 = mybir.dt.float16
F8 = mybir.dt.float8e3     # e3m4
AF = mybir.ActivationFunctionType

_CACHED_NC = None
DEBUG_TAPS = False
SERIAL = False


def build_nc():
    nc = bacc.Bacc("TRN2", target_bir_lowering=False, debug=False,
                   num_devices=NCORES)

    dsl = nc.dram_tensor("dsl", [B, TOK, N, C], F8, kind="ExternalInput")
    # cols 0:C = w_q.T * hd^-0.5, C:2C = w_k.T
    wkqT = nc.dram_tensor("wkqT", [C, 2 * C], F16, kind="ExternalInput")
    wvT = nc.dram_tensor("wvT", [C, C], F16, kind="ExternalInput")
    wpT = nc.dram_tensor("wpT", [C, C], F16, kind="ExternalInput")
    xT = nc.dram_tensor("xT", [C, B * N], F16, kind="ExternalInput")
    xqT = nc.dram_tensor("xqT", [C, B * TOK], F16, kind="ExternalInput")
    bproj = nc.dram_tensor("bproj", [1, C], F16, kind="ExternalInput")
    outp = nc.dram_tensor("outp", [B, TOK, C], F16, kind="ExternalOutput")
    # batch-1 tail tokens, transposed: [cc, ck, i] -> out[1, TOKA+i, 128*ck+cc]
    outpT = nc.dram_tensor("outpT", [128, CK, TOKB], F16,
                           kind="ExternalOutput")

    with tile.TileContext(nc) as tc:
        singles = tc.alloc_tile_pool(name="singles", bufs=1)
        dpool = tc.alloc_tile_pool(name="dpool", bufs=6)
        dtail = tc.alloc_tile_pool(name="dtail", bufs=4)
        smp = tc.alloc_tile_pool(name="smp", bufs=3)
        pss0 = tc.alloc_tile_pool(name="pss0", bufs=1, space="PSUM")
        pss1 = tc.alloc_tile_pool(name="pss1", bufs=1, space="PSUM")
        pssB = tc.alloc_tile_pool(name="pssB", bufs=1, space="PSUM")
        kqps = tc.alloc_tile_pool(name="kqps", bufs=1, space="PSUM")
        apsp = tc.alloc_tile_pool(name="apsp", bufs=1, space="PSUM")
        vpsp = tc.alloc_tile_pool(name="vpsp", bufs=1, space="PSUM")
        fpsp = tc.alloc_tile_pool(name="fpsp", bufs=1, space="PSUM")
        stack = [singles, dpool, dtail, smp, pss0, pss1, pssB, kqps, apsp,
                 vpsp, fpsp]

        # ---- SBUF tiles (all fit; no pool cycling needed) ----
        wkq_sb = singles.tile([128, CK, 2 * C], F16, name="wkq_sb")
        wv_sb = singles.tile([128, CK, C], F16, name="wv_sb")
        wp_sb = singles.tile([128, CK, C], F16, name="wp_sb")
        xT_sb = singles.tile([128, CK, B * N], F16, name="xT_sb")
        xqT_sb = singles.tile([128, CK, B * TOK], F16, name="xqT_sb")
        kT_sb = singles.tile([128, CK, B * N], F16, name="kT_sb")
        qT_sb = singles.tile([128, CK, B * TOK], F16, name="qT_sb")
        v_sb = [singles.tile([128, JT, C], F16, name=f"v{b}") for b in range(B)]
        attnT = [singles.tile([128, JT, H * TOK], F16, name=f"attnT{b}")
                 for b in range(B)]
        hfinT = [singles.tile([128, CK, TOK], F16, name=f"hfinT{b}")
                 for b in range(B)]
        bias16 = singles.tile([1, C], F16, name="bias16")
        ones16 = singles.tile([1, TOK], F16, name="ones16")
        nc.gpsimd.memset(ones16, 1.0)
        out_sb = [singles.tile([TOK, C], F16, name=f"out_sb{b}")
                  for b in range(B)]

        # long-lived PSUM accumulators: d-term + v-term, [c, token] layout.
        # In separate banks: start_tensor_calc marks the whole 2KB zero
        # region pending-zero, so each bank gets exactly one start (per
        # partition half) and one stop. Batch 1 is split into an early part
        # (tokens 0:TOKA) and a small tail part so the final epilogue after
        # the last d chunk is cheap.
        ps_d = [pss0.tile([128, CK, TOK], F32, name="ps_d0"),
                pss1.tile([128, CK, TOKA], F32, name="ps_d1A")]
        ps_dB = pssB.tile([128, CK, TOKB], F32, name="ps_d1B")

        # ---- input DMAs, in intended DMA-engine FIFO order ----
        def load_w_cols(dst, src, c0, c1):
            nc.sync.dma_start(
                out=dst[:, :, c0:c1],
                in_=src.ap()[:, c0:c1].rearrange("(ko ki) co -> ki ko co",
                                                 ki=128))

        # k01, q01 first (gate the first kq piece), then x, then the rest
        load_w_cols(wkq_sb, wkqT, C, C + 256)
        load_w_cols(wkq_sb, wkqT, 0, 256)
        nc.sync.dma_start(
            out=xT_sb[:, :, 0:N],
            in_=xT.ap()[:, 0:N].rearrange("(ko ki) t -> ki ko t", ki=128))
        nc.sync.dma_start(
            out=xqT_sb, in_=xqT.ap().rearrange("(ko ki) t -> ki ko t", ki=128))
        load_w_cols(wkq_sb, wkqT, C + 256, C + 512)
        load_w_cols(wkq_sb, wkqT, 256, 512)
        load_w_cols(wkq_sb, wkqT, C + 512, C + 768)
        load_w_cols(wkq_sb, wkqT, 512, 768)
        nc.sync.dma_start(
            out=wv_sb, in_=wvT.ap().rearrange("(ko ki) co -> ki ko co", ki=128))
        nc.sync.dma_start(
            out=xT_sb[:, :, N:2 * N],
            in_=xT.ap()[:, N:2 * N].rearrange("(ko ki) t -> ki ko t", ki=128))
        nc.sync.dma_start(out=bias16, in_=bproj.ap())
        nc.sync.dma_start(
            out=wp_sb, in_=wpT.ap().rearrange("(ko ki) co -> ki ko co", ki=128))

        # ---- emission helpers ----
        def kq_piece(b, m):
            kps = kqps.tile([128, N + TOK], F32, tag="kqp", name="kqp")
            for kt in range(CK):
                nc.tensor.matmul(
                    kps[:, 0:N], wkq_sb[:, kt, C + m * 128:C + (m + 1) * 128],
                    xT_sb[:, kt, b * N:(b + 1) * N],
                    start=(kt == 0), stop=(kt == CK - 1))
            nc.scalar.copy(out=kT_sb[:, m, b * N:(b + 1) * N], in_=kps[:, 0:N])
            for kt in range(CK):
                nc.tensor.matmul(
                    kps[:, N:N + TOK], wkq_sb[:, kt, m * 128:(m + 1) * 128],
                    xqT_sb[:, kt, b * TOK:(b + 1) * TOK],
                    start=(kt == 0), stop=(kt == CK - 1))
            nc.scalar.copy(out=qT_sb[:, m, b * TOK:(b + 1) * TOK],
                           in_=kps[:, N:N + TOK])

        def attn_piece(b, h):
            p0 = 64 * (h % 2)
            m = h // 2
            aps = apsp.tile([TOK, N], F32, tag="aps", name="aps")
            nc.tensor.matmul(
                aps, qT_sb[p0:p0 + 64, m, b * TOK:(b + 1) * TOK],
                kT_sb[p0:p0 + 64, m, b * N:(b + 1) * N],
                start=True, stop=True)
            # logits are small (|l| < ~4); exp without max-subtraction is safe
            attn16 = smp.tile([TOK, N], F16, tag="attn16", name="attn16")
            rowsum = smp.tile([TOK, 1], F32, tag="rowsum", name="rowsum")
            nc.scalar.activation(out=attn16, in_=aps, func=AF.Exp,
                                 scale=1.0, accum_out=rowsum)
            rinv = smp.tile([TOK, 1], F32, tag="rinv", name="rinv")
            nc.vector.reciprocal(out=rinv, in_=rowsum)
            nc.vector.tensor_scalar_mul(out=attn16, in0=attn16, scalar1=rinv)
            for jt in range(JT):
                for q in range(4):
                    nc.vector.transpose(
                        out=attnT[b][32 * q:32 * (q + 1), jt,
                                     h * TOK:(h + 1) * TOK],
                        in_=attn16[:, jt * 128 + 32 * q:
                                   jt * 128 + 32 * (q + 1)])

        def v_piece(b, jt, ch):
            c0 = 384 * ch
            vps = vpsp.tile([128, 384], F32, tag="vps", name="vps")
            for kt in range(CK):
                nc.tensor.matmul(
                    vps, xT_sb[:, kt, b * N + jt * 128:b * N + (jt + 1) * 128],
                    wv_sb[:, kt, c0:c0 + 384],
                    start=(kt == 0), stop=(kt == CK - 1))
            nc.scalar.copy(out=v_sb[b][:, jt, c0:c0 + 384], in_=vps)

        def d_token(b, il, dt, t):
            # accumulates onto the v-term already in ps_d; the last token's
            # final matmul closes the bank's accumulation group
            if b == 1 and il >= TOKA:
                ps, col, last = ps_dB, il - TOKA, il == TOK - 1
            else:
                ps, col = ps_d[b], il
                last = il == (TOKA - 1 if b == 1 else TOK - 1)
            for ck in range(CK):
                for half in range(2):
                    h = 2 * ck + half
                    for jt in range(JT):
                        nc.tensor.matmul(
                            ps[64 * half:64 * half + 64, ck, col:col + 1],
                            dt[:, t, jt, h * HD:(h + 1) * HD],
                            attnT[b][:, jt, h * TOK + il:h * TOK + il + 1],
                            start=False,
                            stop=(last and ck == CK - 1 and jt == JT - 1),
                            skip_group_check=True)

        def vterm_piece(b):
            # ck==0 jt==0 carries each bank's single start per partition
            # half; later writes zero-fill on first touch, then accumulate
            targets = ([(ps_d[0], 0, TOK)] if b == 0 else
                       [(ps_d[1], 0, TOKA), (ps_dB, TOKA, TOK)])
            for ps, t0, t1 in targets:
                for ck in range(CK):
                    for half in range(2):
                        h = 2 * ck + half
                        for jt in range(JT):
                            nc.tensor.matmul(
                                ps[64 * half:64 * half + 64, ck, :],
                                v_sb[b][:, jt, h * HD:(h + 1) * HD],
                                attnT[b][:, jt, h * TOK + t0:h * TOK + t1],
                                start=(ck == 0 and jt == 0), stop=False,
                                skip_group_check=True)

        def epi_piece(b):
            # normal-orientation projection for batch 0 / batch-1 tokens
            # 0:TOKA; all of it overlaps remaining d streaming
            nt = TOK if b == 0 else TOKA
            nc.vector.tensor_copy(out=hfinT[b][:, :, 0:nt],
                                  in_=ps_d[b])
            fps = fpsp.tile([TOK, C], F32, tag="fps", name="fps")
            # bias via ones-row matmul opens each 2KB zero region
            for lo, hi in ((0, 512), (512, 768)):
                nc.tensor.matmul(fps[0:nt, lo:hi], ones16[:, 0:nt],
                                 bias16[:, lo:hi],
                                 start=True, stop=False, skip_group_check=True)
            for ct in range(CK):
                for lo, hi in ((0, 512), (512, 768)):
                    nc.tensor.matmul(
                        fps[0:nt, lo:hi], hfinT[b][:, ct, 0:nt],
                        wp_sb[:, ct, lo:hi],
                        start=False, stop=(ct == CK - 1),
                        skip_group_check=True)
            # stage PSUM->SBUF split across DVE+ACT (halves the copy latency)
            nc.vector.tensor_copy(out=out_sb[b][0:nt, 0:384],
                                  in_=fps[0:nt, 0:384])
            nc.scalar.copy(out=out_sb[b][0:nt, 384:768],
                           in_=fps[0:nt, 384:768])
            # ACT-side HWDGE queue: doesn't block the SP queue's d streaming
            nc.scalar.dma_start(out=outp.ap()[b, 0:nt], in_=out_sb[b][0:nt])

        def epi_tail_piece():
            # batch-1 tail tokens, transposed projection: tiny free dims so
            # the post-last-chunk critical path is short
            hfB = hfinT[1][:, :, TOKA:TOK]
            nc.vector.tensor_copy(out=hfB, in_=ps_dB)
            oT = vpsp.tile([128, 384], F32, tag="vps", name="outTB")
            oTv = oT[:, 0:CK * TOKB].rearrange("p (ck i) -> p ck i", i=TOKB)
            for co in range(CK):
                nc.tensor.matmul(
                    oTv[:, co, :], bias16[:, 128 * co:128 * (co + 1)],
                    ones16[:, 0:TOKB],
                    start=(co == 0), stop=False, skip_group_check=True)
                for ct in range(CK):
                    nc.tensor.matmul(
                        oTv[:, co, :], wp_sb[:, ct, 128 * co:128 * (co + 1)],
                        hfB[:, ct, :],
                        start=False, stop=(co == CK - 1 and ct == CK - 1),
                        skip_group_check=True)
            oT_sb = singles.tile([128, CK, TOKB], F16, name="oT_sb")
            nc.vector.tensor_copy(out=oT_sb, in_=oTv)
            nc.scalar.dma_start(out=outpT.ap(), in_=oT_sb)

        # ---- phase A: batch-0 attention + v, open batch-0 psum groups ----
        for m in range(CK):
            kq_piece(0, m)
            attn_piece(0, 2 * m)
            attn_piece(0, 2 * m + 1)
        for jt in range(JT):
            for ch in range(2):
                v_piece(0, jt, ch)
        vterm_piece(0)
        # ---- phase A.5: batch-1 attention + v (as d-loop side pieces) ----
        sides = [
            (1, lambda: kq_piece(1, 0)),
            (1, lambda: attn_piece(1, 0)), (1, lambda: attn_piece(1, 1)),
            (2, lambda: kq_piece(1, 1)),
            (2, lambda: attn_piece(1, 2)), (2, lambda: attn_piece(1, 3)),
            (3, lambda: kq_piece(1, 2)),
            (3, lambda: attn_piece(1, 4)), (3, lambda: attn_piece(1, 5)),
            (4, lambda: kq_piece(1, 3)),
            (4, lambda: attn_piece(1, 6)), (4, lambda: attn_piece(1, 7)),
            (5, lambda: kq_piece(1, 4)),
            (5, lambda: attn_piece(1, 8)), (5, lambda: attn_piece(1, 9)),
            (6, lambda: kq_piece(1, 5)),
            (6, lambda: attn_piece(1, 10)), (6, lambda: attn_piece(1, 11)),
            (7, lambda: v_piece(1, 0, 0)), (7, lambda: v_piece(1, 0, 1)),
            (7, lambda: v_piece(1, 1, 0)), (7, lambda: v_piece(1, 1, 1)),
            (8, lambda: vterm_piece(1)), (8, lambda: epi_piece(0)),
            (14, lambda: epi_piece(1)),
        ]
        if SERIAL:
            for m in range(CK):
                kq_piece(1, m)
                attn_piece(1, 2 * m)
                attn_piece(1, 2 * m + 1)
            for jt in range(JT):
                for ch in range(2):
                    v_piece(1, jt, ch)
            vterm_piece(1)
            sides = []
        emitted = 0
        # batch-1's final 4 tokens stream as single-token DMAs so their PE
        # work overlaps the incoming transfers instead of trailing them
        chunks = [(0, ic0, NTOK) for ic0 in range(0, TOK, NTOK)]
        chunks += [(1, ic0, NTOK) for ic0 in range(0, TOK - NTOK, NTOK)]
        chunks += [(1, ic0, 1) for ic0 in range(TOK - NTOK, TOK)]
        for ci, (b, ic0, n) in enumerate(chunks):
            while emitted < len(sides) and sides[emitted][0] <= ci:
                sides[emitted][1]()
                emitted += 1
            pool = dpool if n == NTOK else dtail
            dt = pool.tile([128, n, JT, C], F8,
                           name="d_tile" if n == NTOK else "d_tail")
            nc.sync.dma_start(
                out=dt,
                in_=dsl.ap()[b, ic0:ic0 + n].rearrange(
                    "t (jt p) c -> p t jt c", p=128))
            for t in range(n):
                d_token(b, ic0 + t, dt, t)
        while emitted < len(sides):
            sides[emitted][1]()
            emitted += 1

        # ---- tail: batch-1 tail-token epilogue ----
        if SERIAL:
            epi_piece(0)
            epi_piece(1)
        epi_tail_piece()

        if DEBUG_TAPS:
            d_kT = nc.dram_tensor("d_kT", [128, CK, B * N], F16,
                                  kind="ExternalOutput")
            d_qT = nc.dram_tensor("d_qT", [128, CK, B * TOK], F16,
                                  kind="ExternalOutput")
            d_attnT = nc.dram_tensor("d_attnT", [B, 128, JT, H * TOK], F16,
                                     kind="ExternalOutput")
            d_hfinT = nc.dram_tensor("d_hfinT", [B, 128, CK, TOK], F16,
                                     kind="ExternalOutput")
            d_v = nc.dram_tensor("d_v", [B, 128, JT, C], F16,
                                 kind="ExternalOutput")
            nc.sync.dma_start(out=d_kT.ap(), in_=kT_sb)
            nc.sync.dma_start(out=d_qT.ap(), in_=qT_sb)
            for b in range(B):
                nc.sync.dma_start(out=d_attnT.ap()[b], in_=attnT[b])
                nc.sync.dma_start(out=d_hfinT.ap()[b], in_=hfinT[b])
                nc.sync.dma_start(out=d_v.ap()[b], in_=v_sb[b])

        for p in reversed(stack):
            p.release()

    nc.compile()
    return nc


def make_in_maps(x, d, w_qkv, w_proj, b_proj):
    x = np.asarray(x, dtype=np.float32)
    w_qkv = np.asarray(w_qkv, dtype=np.float32)
    w_proj = np.asarray(w_proj, dtype=np.float32)
    b_proj = np.asarray(b_proj, dtype=np.float32)

    scale = HD ** -0.5
    wq = np.ascontiguousarray((w_qkv[0:C] * scale).T)
    wk = np.ascontiguousarray(w_qkv[C:2 * C].T)
    wkqT = np.concatenate([wq, wk], axis=1).astype(np.float16)   # [C, 2C]
    wvT = np.ascontiguousarray(w_qkv[2 * C:3 * C].T).astype(np.float16)
    wpT = np.ascontiguousarray(w_proj.T).astype(np.float16)
    xTf = np.ascontiguousarray(
        x.reshape(B * N, C).T).astype(np.float16)                # [C, B*N]
    d8 = np.asarray(d, dtype=np.float32).astype(ml_dtypes.float8_e3m4)

    in_maps = []
    for c in range(NCORES):
        i0 = c * TOK
        xq = x[:, i0:i0 + TOK, :].reshape(B * TOK, C)
        in_maps.append({
            "dsl": np.ascontiguousarray(d8[:, i0:i0 + TOK]),
            "wkqT": wkqT,
            "wvT": wvT,
            "wpT": wpT,
            "xT": xTf,
            "xqT": np.ascontiguousarray(xq.T).astype(np.float16),
            "bproj": b_proj.reshape(1, C).astype(np.float16),
        })
    return in_maps


def kernel(x, d, w_qkv, w_proj, b_proj):
    global _CACHED_NC
    if _CACHED_NC is None:
        _CACHED_NC = build_nc()
    nc = _CACHED_NC

    in_maps = make_in_maps(x, d, w_qkv, w_proj, b_proj)
    res = run_bass_kernel_spmd(nc, in_maps, core_ids=list(range(NCORES)))

    out = np.empty((B, N, C), dtype=np.float32)
    for c in range(NCORES):
        i0 = c * TOK
        out[:, i0:i0 + TOK, :] = \
            np.asarray(res.results[c]["outp"]).astype(np.float32)
        # batch-1 tail tokens come back transposed: [cc, ck, i]
        oT = np.asarray(res.results[c]["outpT"]).astype(np.float32)
        out[1, i0 + TOKA:i0 + TOK, :] = \
            oT.transpose(2, 1, 0).reshape(TOKB, C)
    return out


# revision 53
# speedup vs baseline: 1.0162x; 1.0162x over previous
"""Trainium2 Bass kernel for nn_Attention_D (pairwise-bias attention).

Problem: B=2, N=256, C=768, H=12, hd=64
  qkv = x @ w_qkv.T ; attn = softmax(q k^T * hd^-0.5)
  out = attn @ v + einsum('bhij,bhijd->bhid', attn, dh); out @ w_proj.T + b

d [B, N, N, C] dominates; the kernel is DMA-bound (global DMA pipe
~360 B/ns per core). Query rows are sharded across the 8 cores (32 per
batch per core); d streams in as float8_e3m4 (1 byte, ~1e-2 rel err vs
the 2e-2 gate; fp8 weights/attn tested and rejected), everything else
fp16. Per-core DMA ~51.4 us of the ~58.7 us total.

The d-term out2[h,i,c] = sum_j attn[h,i,j] * d[i,j,c] is computed entirely
on PE in transposed form: per token i, per 128-wide c-chunk ck and 64-wide
half (head h = 2*ck + half), a tiny matmul
    psum[c, i-col] += d_i[j, c-block].T(lhsT, e3m4) @ attnT[:, h-col](fp16)
accumulates the diagonal-block result directly into a [c, token] PSUM
layout (out free size 1 -> ~no PE time; PE reads e3m4 straight from the
DMA tile; mixed e3m4 x fp16 matmul validated on HW). The v-term
(v.T as lhsT, attnT as rhs) accumulates into the same PSUM region, so the
epilogue is a single PSUM->SBUF fp16 copy per (b, ck) producing hfinT in
exactly the lhsT layout the final projection needs. No DVE d-path, no
diagonal extraction, no transposes of the output.
"""

import numpy as np
import ml_dtypes

import concourse.bass as bass
import concourse.bacc as bacc
import concourse.mybir as mybir
import concourse.tile as tile
from concourse.bass_utils import run_bass_kernel_spmd

B, N, C = 2, 256, 768
H, HD = 12, 64
NCORES = 8
TOK = N // NCORES          # 32 own query rows per batch per core
CK = C // 128              # 6 c-chunks
JT = N // 128              # 2 j partition tiles
NTOK = 4                   # tokens per d DMA chunk
TOKA = 24                  # batch-1 tokens projected early (rest: tail path)
TOKB = TOK - TOKA
F32 = mybir.dt.float32
F16 = mybir.dt.float16
F8 = mybir.dt.float8e3     # e3m4
AF = mybir.ActivationFunctionType

_CACHED_NC = None
DEBUG_TAPS = False
SERIAL = False


def build_nc():
    nc = bacc.Bacc("TRN2", target_bir_lowering=False, debug=False,
                   num_devices=NCORES)

    dsl = nc.dram_tensor("dsl", [B, TOK, N, C], F8, kind="ExternalInput")
    # cols 0:C = w_q.T * hd^-0.5, C:2C = w_k.T
    wkqT = nc.dram_tensor("wkqT", [C, 2 * C], F16, kind="ExternalInput")
    wvT = nc.dram_tensor("wvT", [C, C], F16, kind="ExternalInput")
    wpT = nc.dram_tensor("wpT", [C, C], F16, kind="ExternalInput")
    xT = nc.dram_tensor("xT", [C, B * N], F16, kind="ExternalInput")
    xqT = nc.dram_tensor("xqT", [C, B * TOK], F16, kind="ExternalInput")
    bproj = nc.dram_tensor("bproj", [1, C], F16, kind="ExternalInput")
    outp = nc.dram_tensor("outp", [B, TOK, C], F16, kind="ExternalOutput")
    # batch-1 tail tokens, transposed: [cc, ck, i] -> out[1, TOKA+i, 128*ck+cc]
    outpT = nc.dram_tensor("outpT", [128, CK, TOKB], F16,
                           kind="ExternalOutput")

    with tile.TileContext(nc) as tc:
        singles = tc.alloc_tile_pool(name="singles", bufs=1)
        dpool = tc.alloc_tile_pool(name="dpool", bufs=6)
        smp = tc.alloc_tile_pool(name="smp", bufs=3)
        pss0 = tc.alloc_tile_pool(name="pss0", bufs=1, space="PSUM")
        pss1 = tc.alloc_tile_pool(name="pss1", bufs=1, space="PSUM")
        pssB = tc.alloc_tile_pool(name="pssB", bufs=1, space="PSUM")
        kqps = tc.alloc_tile_pool(name="kqps", bufs=1, space="PSUM")
        apsp = tc.alloc_tile_pool(name="apsp", bufs=1, space="PSUM")
        vpsp = tc.alloc_tile_pool(name="vpsp", bufs=1, space="PSUM")
        fpsp = tc.alloc_tile_pool(name="fpsp", bufs=1, space="PSUM")
        stack = [singles, dpool, smp, pss0, pss1, pssB, kqps, apsp,
                 vpsp, fpsp]

        # ---- SBUF tiles (all fit; no pool cycling needed) ----
        wkq_sb = singles.tile([128, CK, 2 * C], F16, name="wkq_sb")
        wv_sb = singles.tile([128, CK, C], F16, name="wv_sb")
        wp_sb = singles.tile([128, CK, C], F16, name="wp_sb")
        xT_sb = singles.tile([128, CK, B * N], F16, name="xT_sb")
        xqT_sb = singles.tile([128, CK, B * TOK], F16, name="xqT_sb")
        kT_sb = singles.tile([128, CK, B * N], F16, name="kT_sb")
        qT_sb = singles.tile([128, CK, B * TOK], F16, name="qT_sb")
        v_sb = [singles.tile([128, JT, C], F16, name=f"v{b}") for b in range(B)]
        attnT = [singles.tile([128, JT, H * TOK], F16, name=f"attnT{b}")
                 for b in range(B)]
        hfinT = [singles.tile([128, CK, TOK], F16, name=f"hfinT{b}")
                 for b in range(B)]
        bias16 = singles.tile([1, C], F16, name="bias16")
        ones16 = singles.tile([1, TOK], F16, name="ones16")
        nc.gpsimd.memset(ones16, 1.0)
        out_sb = [singles.tile([TOK, C], F16, name=f"out_sb{b}")
                  for b in range(B)]

        # long-lived PSUM accumulators: d-term + v-term, [c, token] layout.
        # In separate banks: start_tensor_calc marks the whole 2KB zero
        # region pending-zero, so each bank gets exactly one start (per
        # partition half) and one stop. Batch 1 is split into an early part
        # (tokens 0:TOKA) and a small tail part so the final epilogue after
        # the last d chunk is cheap.
        ps_d = [pss0.tile([128, CK, TOK], F32, name="ps_d0"),
                pss1.tile([128, CK, TOKA], F32, name="ps_d1A")]
        ps_dB = pssB.tile([128, CK, TOKB], F32, name="ps_d1B")

        # ---- input DMAs, in intended DMA-engine FIFO order ----
        def load_w_cols(dst, src, c0, c1):
            nc.sync.dma_start(
                out=dst[:, :, c0:c1],
                in_=src.ap()[:, c0:c1].rearrange("(ko ki) co -> ki ko co",
                                                 ki=128))

        # k01, q01 first (gate the first kq piece), then x, then the rest
        load_w_cols(wkq_sb, wkqT, C, C + 256)
        load_w_cols(wkq_sb, wkqT, 0, 256)
        nc.sync.dma_start(
            out=xT_sb[:, :, 0:N],
            in_=xT.ap()[:, 0:N].rearrange("(ko ki) t -> ki ko t", ki=128))
        nc.sync.dma_start(
            out=xqT_sb, in_=xqT.ap().rearrange("(ko ki) t -> ki ko t", ki=128))
        load_w_cols(wkq_sb, wkqT, C + 256, C + 512)
        load_w_cols(wkq_sb, wkqT, 256, 512)
        load_w_cols(wkq_sb, wkqT, C + 512, C + 768)
        load_w_cols(wkq_sb, wkqT, 512, 768)
        nc.sync.dma_start(
            out=wv_sb, in_=wvT.ap().rearrange("(ko ki) co -> ki ko co", ki=128))
        nc.sync.dma_start(
            out=xT_sb[:, :, N:2 * N],
            in_=xT.ap()[:, N:2 * N].rearrange("(ko ki) t -> ki ko t", ki=128))
        nc.sync.dma_start(out=bias16, in_=bproj.ap())
        nc.sync.dma_start(
            out=wp_sb, in_=wpT.ap().rearrange("(ko ki) co -> ki ko co", ki=128))

        # ---- emission helpers ----
        def kq_piece(b, m):
            kps = kqps.tile([128, N + TOK], F32, tag="kqp", name="kqp")
            for kt in range(CK):
                nc.tensor.matmul(
                    kps[:, 0:N], wkq_sb[:, kt, C + m * 128:C + (m + 1) * 128],
                    xT_sb[:, kt, b * N:(b + 1) * N],
                    start=(kt == 0), stop=(kt == CK - 1))
            nc.scalar.copy(out=kT_sb[:, m, b * N:(b + 1) * N], in_=kps[:, 0:N])
            for kt in range(CK):
                nc.tensor.matmul(
                    kps[:, N:N + TOK], wkq_sb[:, kt, m * 128:(m + 1) * 128],
                    xqT_sb[:, kt, b * TOK:(b + 1) * TOK],
                    start=(kt == 0), stop=(kt == CK - 1))
            nc.scalar.copy(out=qT_sb[:, m, b * TOK:(b + 1) * TOK],
                           in_=kps[:, N:N + TOK])

        def attn_piece(b, h):
            p0 = 64 * (h % 2)
            m = h // 2
            aps = apsp.tile([TOK, N], F32, tag="aps", name="aps")
            nc.tensor.matmul(
                aps, qT_sb[p0:p0 + 64, m, b * TOK:(b + 1) * TOK],
                kT_sb[p0:p0 + 64, m, b * N:(b + 1) * N],
                start=True, stop=True)
            # logits are small (|l| < ~4); exp without max-subtraction is safe
            attn16 = smp.tile([TOK, N], F16, tag="attn16", name="attn16")
            rowsum = smp.tile([TOK, 1], F32, tag="rowsum", name="rowsum")
            nc.scalar.activation(out=attn16, in_=aps, func=AF.Exp,
                                 scale=1.0, accum_out=rowsum)
            rinv = smp.tile([TOK, 1], F32, tag="rinv", name="rinv")
            nc.vector.reciprocal(out=rinv, in_=rowsum)
            nc.vector.tensor_scalar_mul(out=attn16, in0=attn16, scalar1=rinv)
            for jt in range(JT):
                for q in range(4):
                    nc.vector.transpose(
                        out=attnT[b][32 * q:32 * (q + 1), jt,
                                     h * TOK:(h + 1) * TOK],
                        in_=attn16[:, jt * 128 + 32 * q:
                                   jt * 128 + 32 * (q + 1)])

        def v_piece(b, jt, ch):
            c0 = 384 * ch
            vps = vpsp.tile([128, 384], F32, tag="vps", name="vps")
            for kt in range(CK):
                nc.tensor.matmul(
                    vps, xT_sb[:, kt, b * N + jt * 128:b * N + (jt + 1) * 128],
                    wv_sb[:, kt, c0:c0 + 384],
                    start=(kt == 0), stop=(kt == CK - 1))
            nc.scalar.copy(out=v_sb[b][:, jt, c0:c0 + 384], in_=vps)

        def d_token(b, il, dt, t):
            # accumulates onto the v-term already in ps_d; the last token's
            # final matmul closes the bank's accumulation group
            if b == 1 and il >= TOKA:
                ps, col, last = ps_dB, il - TOKA, il == TOK - 1
            else:
                ps, col = ps_d[b], il
                last = il == (TOKA - 1 if b == 1 else TOK - 1)
            for ck in range(CK):
                for half in range(2):
                    h = 2 * ck + half
                    for jt in range(JT):
                        nc.tensor.matmul(
                            ps[64 * half:64 * half + 64, ck, col:col + 1],
                            dt[:, t, jt, h * HD:(h + 1) * HD],
                            attnT[b][:, jt, h * TOK + il:h * TOK + il + 1],
                            start=False,
                            stop=(last and ck == CK - 1 and jt == JT - 1),
                            skip_group_check=True)

        def vterm_piece(b):
            # ck==0 jt==0 carries each bank's single start per partition
            # half; later writes zero-fill on first touch, then accumulate
            targets = ([(ps_d[0], 0, TOK)] if b == 0 else
                       [(ps_d[1], 0, TOKA), (ps_dB, TOKA, TOK)])
            for ps, t0, t1 in targets:
                for ck in range(CK):
                    for half in range(2):
                        h = 2 * ck + half
                        for jt in range(JT):
                            nc.tensor.matmul(
                                ps[64 * half:64 * half + 64, ck, :],
                                v_sb[b][:, jt, h * HD:(h + 1) * HD],
                                attnT[b][:, jt, h * TOK + t0:h * TOK + t1],
                                start=(ck == 0 and jt == 0), stop=False,
                                skip_group_check=True)

        def epi_piece(b):
            # normal-orientation projection for batch 0 / batch-1 tokens
            # 0:TOKA; all of it overlaps remaining d streaming
            nt = TOK if b == 0 else TOKA
            nc.vector.tensor_copy(out=hfinT[b][:, :, 0:nt],
                                  in_=ps_d[b])
            fps = fpsp.tile([TOK, C], F32, tag="fps", name="fps")
            # bias via ones-row matmul opens each 2KB zero region
            for lo, hi in ((0, 512), (512, 768)):
                nc.tensor.matmul(fps[0:nt, lo:hi], ones16[:, 0:nt],
                                 bias16[:, lo:hi],
                                 start=True, stop=False, skip_group_check=True)
            for ct in range(CK):
                for lo, hi in ((0, 512), (512, 768)):
                    nc.tensor.matmul(
                        fps[0:nt, lo:hi], hfinT[b][:, ct, 0:nt],
                        wp_sb[:, ct, lo:hi],
                        start=False, stop=(ct == CK - 1),
                        skip_group_check=True)
            # stage PSUM->SBUF split across DVE+ACT (halves the copy latency)
            nc.vector.tensor_copy(out=out_sb[b][0:nt, 0:384],
                                  in_=fps[0:nt, 0:384])
            nc.scalar.copy(out=out_sb[b][0:nt, 384:768],
                           in_=fps[0:nt, 384:768])
            # ACT-side HWDGE queue: doesn't block the SP queue's d streaming
            nc.scalar.dma_start(out=outp.ap()[b, 0:nt], in_=out_sb[b][0:nt])

        def epi_tail_piece():
            # batch-1 tail tokens, transposed projection: tiny free dims so
            # the post-last-chunk critical path is short
            hfB = hfinT[1][:, :, TOKA:TOK]
            nc.vector.tensor_copy(out=hfB, in_=ps_dB)
            oT = vpsp.tile([128, 384], F32, tag="vps", name="outTB")
            oTv = oT[:, 0:CK * TOKB].rearrange("p (ck i) -> p ck i", i=TOKB)
            for co in range(CK):
                nc.tensor.matmul(
                    oTv[:, co, :], bias16[:, 128 * co:128 * (co + 1)],
                    ones16[:, 0:TOKB],
                    start=(co == 0), stop=False, skip_group_check=True)
                for ct in range(CK):
                    nc.tensor.matmul(
                        oTv[:, co, :], wp_sb[:, ct, 128 * co:128 * (co + 1)],
                        hfB[:, ct, :],
                        start=False, stop=(co == CK - 1 and ct == CK - 1),
                        skip_group_check=True)
            oT_sb = singles.tile([128, CK, TOKB], F16, name="oT_sb")
            nc.vector.tensor_copy(out=oT_sb, in_=oTv)
            nc.scalar.dma_start(out=outpT.ap(), in_=oT_sb)

        # ---- phase A: batch-0 attention + v, open batch-0 psum groups ----
        for m in range(CK):
            kq_piece(0, m)
            attn_piece(0, 2 * m)
            attn_piece(0, 2 * m + 1)
        for jt in range(JT):
            for ch in range(2):
                v_piece(0, jt, ch)
        vterm_piece(0)
        # ---- phase A.5: batch-1 attention + v (as d-loop side pieces) ----
        sides = [
            (1, lambda: kq_piece(1, 0)),
            (1, lambda: attn_piece(1, 0)), (1, lambda: attn_piece(1, 1)),
            (2, lambda: kq_piece(1, 1)),
            (2, lambda: attn_piece(1, 2)), (2, lambda: attn_piece(1, 3)),
            (3, lambda: kq_piece(1, 2)),
            (3, lambda: attn_piece(1, 4)), (3, lambda: attn_piece(1, 5)),
            (4, lambda: kq_piece(1, 3)),
            (4, lambda: attn_piece(1, 6)), (4, lambda: attn_piece(1, 7)),
            (5, lambda: kq_piece(1, 4)),
            (5, lambda: attn_piece(1, 8)), (5, lambda: attn_piece(1, 9)),
            (6, lambda: kq_piece(1, 5)),
            (6, lambda: attn_piece(1, 10)), (6, lambda: attn_piece(1, 11)),
            (7, lambda: v_piece(1, 0, 0)), (7, lambda: v_piece(1, 0, 1)),
            (7, lambda: v_piece(1, 1, 0)), (7, lambda: v_piece(1, 1, 1)),
            (8, lambda: vterm_piece(1)), (8, lambda: epi_piece(0)),
            (14, lambda: epi_piece(1)),
        ]
        if SERIAL:
            for m in range(CK):
                kq_piece(1, m)
                attn_piece(1, 2 * m)
                attn_piece(1, 2 * m + 1)
            for jt in range(JT):
                for ch in range(2):
                    v_piece(1, jt, ch)
            vterm_piece(1)
            sides = []
        emitted = 0
        chunks = [(b, ic0) for b in range(B) for ic0 in range(0, TOK, NTOK)]
        for ci, (b, ic0) in enumerate(chunks):
            while emitted < len(sides) and sides[emitted][0] <= ci:
                sides[emitted][1]()
                emitted += 1
            dt = dpool.tile([128, NTOK, JT, C], F8, name="d_tile")
            nc.sync.dma_start(
                out=dt,
                in_=dsl.ap()[b, ic0:ic0 + NTOK].rearrange(
                    "t (jt p) c -> p t jt c", p=128))
            for t in range(NTOK):
                d_token(b, ic0 + t, dt, t)
        while emitted < len(sides):
            sides[emitted][1]()
            emitted += 1

        # ---- tail: batch-1 tail-token epilogue ----
        if SERIAL:
            epi_piece(0)
            epi_piece(1)
        epi_tail_piece()

        if DEBUG_TAPS:
            d_kT = nc.dram_tensor("d_kT", [128, CK, B * N], F16,
                                  kind="ExternalOutput")
            d_qT = nc.dram_tensor("d_qT", [128, CK, B * TOK], F16,
                                  kind="ExternalOutput")
            d_attnT = nc.dram_tensor("d_attnT", [B, 128, JT, H * TOK], F16,
                                     kind="ExternalOutput")
            d_hfinT = nc.dram_tensor("d_hfinT", [B, 128, CK, TOK], F16,
                                     kind="ExternalOutput")
            d_v = nc.dram_tensor("d_v", [B, 128, JT, C], F16,
                                 kind="ExternalOutput")
            nc.sync.dma_start(out=d_kT.ap(), in_=kT_sb)
            nc.sync.dma_start(out=d_qT.ap(), in_=qT_sb)
            for b in range(B):
                nc.sync.dma_start(out=d_attnT.ap()[b], in_=attnT[b])
                nc.sync.dma_start(out=d_hfinT.ap()[b], in_=hfinT[b])
                nc.sync.dma_start(out=d_v.ap()[b], in_=v_sb[b])

        for p in reversed(stack):
            p.release()

    nc.compile()
    return nc


def make_in_maps(x, d, w_qkv, w_proj, b_proj):
    x = np.asarray(x, dtype=np.float32)
    w_qkv = np.asarray(w_qkv, dtype=np.float32)
    w_proj = np.asarray(w_proj, dtype=np.float32)
    b_proj = np.asarray(b_proj, dtype=np.float32)

    scale = HD ** -0.5
    wq = np.ascontiguousarray((w_qkv[0:C] * scale).T)
    wk = np.ascontiguousarray(w_qkv[C:2 * C].T)
    wkqT = np.concatenate([wq, wk], axis=1).astype(np.float16)   # [C, 2C]
    wvT = np.ascontiguousarray(w_qkv[2 * C:3 * C].T).astype(np.float16)
    wpT = np.ascontiguousarray(w_proj.T).astype(np.float16)
    xTf = np.ascontiguousarray(
        x.reshape(B * N, C).T).astype(np.float16)                # [C, B*N]
    d8 = np.asarray(d, dtype=np.float32).astype(ml_dtypes.float8_e3m4)

    in_maps = []
    for c in range(NCORES):
        i0 = c * TOK
        xq = x[:, i0:i0 + TOK, :].reshape(B * TOK, C)
        in_maps.append({
            "dsl": np.ascontiguousarray(d8[:, i0:i0 + TOK]),
            "wkqT": wkqT,
            "wvT": wvT,
            "wpT": wpT,
            "xT": xTf,
            "xqT": np.ascontiguousarray(xq.T).astype(np.float16),
            "bproj": b_proj.reshape(1, C).astype(np.float16),
        })
    return in_maps


def kernel(x, d, w_qkv, w_proj, b_proj):
    global _CACHED_NC
    if _CACHED_NC is None:
        _CACHED_NC = build_nc()
    nc = _CACHED_NC

    in_maps = make_in_maps(x, d, w_qkv, w_proj, b_proj)
    res = run_bass_kernel_spmd(nc, in_maps, core_ids=list(range(NCORES)))

    out = np.empty((B, N, C), dtype=np.float32)
    for c in range(NCORES):
        i0 = c * TOK
        out[:, i0:i0 + TOK, :] = \
            np.asarray(res.results[c]["outp"]).astype(np.float32)
        # batch-1 tail tokens come back transposed: [cc, ck, i]
        oT = np.asarray(res.results[c]["outpT"]).astype(np.float32)
        out[1, i0 + TOKA:i0 + TOK, :] = \
            oT.transpose(2, 1, 0).reshape(TOKB, C)
    return out


# revision 58
# speedup vs baseline: 1.0187x; 1.0025x over previous
"""Trainium2 Bass kernel for nn_Attention_D (pairwise-bias attention).

Problem: B=2, N=256, C=768, H=12, hd=64
  qkv = x @ w_qkv.T ; attn = softmax(q k^T * hd^-0.5)
  out = attn @ v + einsum('bhij,bhijd->bhid', attn, dh); out @ w_proj.T + b

d [B, N, N, C] dominates; the kernel is DMA-bound (global DMA pipe
~360 B/ns per core). Query rows are sharded across the 8 cores (32 per
batch per core); d streams in as float8_e3m4 (1 byte, ~1e-2 rel err vs
the 2e-2 gate; fp8 weights/attn tested and rejected), everything else
fp16. Per-core DMA ~51.4 us of the ~58.7 us total.

The d-term out2[h,i,c] = sum_j attn[h,i,j] * d[i,j,c] is computed entirely
on PE in transposed form: per token i, per 128-wide c-chunk ck and 64-wide
half (head h = 2*ck + half), a tiny matmul
    psum[c, i-col] += d_i[j, c-block].T(lhsT, e3m4) @ attnT[:, h-col](fp16)
accumulates the diagonal-block result directly into a [c, token] PSUM
layout (out free size 1 -> ~no PE time; PE reads e3m4 straight from the
DMA tile; mixed e3m4 x fp16 matmul validated on HW). The v-term
(v.T as lhsT, attnT as rhs) accumulates into the same PSUM region, so the
epilogue is a single PSUM->SBUF fp16 copy per (b, ck) producing hfinT in
exactly the lhsT layout the final projection needs. No DVE d-path, no
diagonal extraction, no transposes of the output.
"""

import numpy as np
import ml_dtypes

import concourse.bass as bass
import concourse.bacc as bacc
import concourse.mybir as mybir
import concourse.tile as tile
from concourse.bass_utils import run_bass_kernel_spmd

B, N, C = 2, 256, 768
H, HD = 12, 64
NCORES = 8
TOK = N // NCORES          # 32 own query rows per batch per core
CK = C // 128              # 6 c-chunks
JT = N // 128              # 2 j partition tiles
NTOK = 4                   # tokens per d DMA chunk
TOKA = 24                  # batch-1 tokens projected early (rest: tail path)
TOKB = TOK - TOKA
F32 = mybir.dt.float32
F16 = mybir.dt.float16
F8 = mybir.dt.float8e3     # e3m4
AF = mybir.ActivationFunctionType

_CACHED_NC = None
DEBUG_TAPS = False
SERIAL = False


def build_nc():
    nc = bacc.Bacc("TRN2", target_bir_lowering=False, debug=False,
                   num_devices=NCORES)

    dsl = nc.dram_tensor("dsl", [B, TOK, N, C], F8, kind="ExternalInput")
    # cols 0:C = w_q.T * hd^-0.5, C:2C = w_k.T
    wkqT = nc.dram_tensor("wkqT", [C, 2 * C], F16, kind="ExternalInput")
    wvT = nc.dram_tensor("wvT", [C, C], F16, kind="ExternalInput")
    wpT = nc.dram_tensor("wpT", [C, C], F16, kind="ExternalInput")
    xT = nc.dram_tensor("xT", [C, B * N], F16, kind="ExternalInput")
    xqT = nc.dram_tensor("xqT", [C, B * TOK], F16, kind="ExternalInput")
    bproj = nc.dram_tensor("bproj", [1, C], F16, kind="ExternalInput")
    outp = nc.dram_tensor("outp", [B, TOK, C], F16, kind="ExternalOutput")
    # batch-1 tail tokens, transposed: [cc, ck, i] -> out[1, TOKA+i, 128*ck+cc]
    outpT = nc.dram_tensor("outpT", [128, CK, TOKB], F16,
                           kind="ExternalOutput")

    with tile.TileContext(nc) as tc:
        singles = tc.alloc_tile_pool(name="singles", bufs=1)
        dpool = tc.alloc_tile_pool(name="dpool", bufs=6)
        pss0 = tc.alloc_tile_pool(name="pss0", bufs=1, space="PSUM")
        pss1 = tc.alloc_tile_pool(name="pss1", bufs=1, space="PSUM")
        pssB = tc.alloc_tile_pool(name="pssB", bufs=1, space="PSUM")
        vpsp = tc.alloc_tile_pool(name="vpsp", bufs=1, space="PSUM")
        fpsp = tc.alloc_tile_pool(name="fpsp", bufs=1, space="PSUM")
        # attention-phase pools go on top of the pool stack: they are
        # released mid-stream (LIFO order) once batch-1 attention is done
        smp = tc.alloc_tile_pool(name="smp", bufs=3)
        kqps = tc.alloc_tile_pool(name="kqps", bufs=1, space="PSUM")
        apsp = tc.alloc_tile_pool(name="apsp", bufs=1, space="PSUM")
        stack = [singles, dpool, pss0, pss1, pssB, vpsp, fpsp]

        # ---- SBUF tiles (all fit; no pool cycling needed) ----
        wkq_sb = singles.tile([128, CK, 2 * C], F16, name="wkq_sb")
        wv_sb = singles.tile([128, CK, C], F16, name="wv_sb")
        wp_sb = singles.tile([128, CK, C], F16, name="wp_sb")
        xT_sb = singles.tile([128, CK, B * N], F16, name="xT_sb")
        xqT_sb = singles.tile([128, CK, B * TOK], F16, name="xqT_sb")
        kT_sb = singles.tile([128, CK, B * N], F16, name="kT_sb")
        qT_sb = singles.tile([128, CK, B * TOK], F16, name="qT_sb")
        v_sb = [singles.tile([128, JT, C], F16, name=f"v{b}") for b in range(B)]
        attnT = [singles.tile([128, JT, H * TOK], F16, name=f"attnT{b}")
                 for b in range(B)]
        hfinT = [singles.tile([128, CK, TOK], F16, name=f"hfinT{b}")
                 for b in range(B)]
        bias16 = singles.tile([1, C], F16, name="bias16")
        ones16 = singles.tile([1, TOK], F16, name="ones16")
        nc.gpsimd.memset(ones16, 1.0)
        out_sb = [singles.tile([TOK, C], F16, name=f"out_sb{b}")
                  for b in range(B)]

        # long-lived PSUM accumulators: d-term + v-term, [c, token] layout.
        # In separate banks: start_tensor_calc marks the whole 2KB zero
        # region pending-zero, so each bank gets exactly one start (per
        # partition half) and one stop. Batch 1 is split into an early part
        # (tokens 0:TOKA) and a small tail part so the final epilogue after
        # the last d chunk is cheap.
        ps_d = [pss0.tile([128, CK, TOK], F32, name="ps_d0"),
                pss1.tile([128, CK, TOKA], F32, name="ps_d1A")]
        ps_dB = pssB.tile([128, CK, TOKB], F32, name="ps_d1B")

        # ---- input DMAs, in intended DMA-engine FIFO order ----
        def load_w_cols(dst, src, c0, c1):
            nc.sync.dma_start(
                out=dst[:, :, c0:c1],
                in_=src.ap()[:, c0:c1].rearrange("(ko ki) co -> ki ko co",
                                                 ki=128))

        # k01, q01 first (gate the first kq piece), then x, then the rest
        load_w_cols(wkq_sb, wkqT, C, C + 256)
        load_w_cols(wkq_sb, wkqT, 0, 256)
        nc.sync.dma_start(
            out=xT_sb[:, :, 0:N],
            in_=xT.ap()[:, 0:N].rearrange("(ko ki) t -> ki ko t", ki=128))
        nc.sync.dma_start(
            out=xqT_sb, in_=xqT.ap().rearrange("(ko ki) t -> ki ko t", ki=128))
        load_w_cols(wkq_sb, wkqT, C + 256, C + 512)
        load_w_cols(wkq_sb, wkqT, 256, 512)
        load_w_cols(wkq_sb, wkqT, C + 512, C + 768)
        load_w_cols(wkq_sb, wkqT, 512, 768)
        nc.sync.dma_start(
            out=wv_sb, in_=wvT.ap().rearrange("(ko ki) co -> ki ko co", ki=128))
        nc.sync.dma_start(
            out=xT_sb[:, :, N:2 * N],
            in_=xT.ap()[:, N:2 * N].rearrange("(ko ki) t -> ki ko t", ki=128))
        nc.sync.dma_start(out=bias16, in_=bproj.ap())
        nc.sync.dma_start(
            out=wp_sb, in_=wpT.ap().rearrange("(ko ki) co -> ki ko co", ki=128))

        # ---- emission helpers ----
        def kq_piece(b, m):
            kps = kqps.tile([128, N + TOK], F32, tag="kqp", name="kqp")
            for kt in range(CK):
                nc.tensor.matmul(
                    kps[:, 0:N], wkq_sb[:, kt, C + m * 128:C + (m + 1) * 128],
                    xT_sb[:, kt, b * N:(b + 1) * N],
                    start=(kt == 0), stop=(kt == CK - 1))
            nc.scalar.copy(out=kT_sb[:, m, b * N:(b + 1) * N], in_=kps[:, 0:N])
            for kt in range(CK):
                nc.tensor.matmul(
                    kps[:, N:N + TOK], wkq_sb[:, kt, m * 128:(m + 1) * 128],
                    xqT_sb[:, kt, b * TOK:(b + 1) * TOK],
                    start=(kt == 0), stop=(kt == CK - 1))
            nc.scalar.copy(out=qT_sb[:, m, b * TOK:(b + 1) * TOK],
                           in_=kps[:, N:N + TOK])

        def attn_piece(b, h):
            p0 = 64 * (h % 2)
            m = h // 2
            aps = apsp.tile([TOK, N], F32, tag="aps", name="aps")
            nc.tensor.matmul(
                aps, qT_sb[p0:p0 + 64, m, b * TOK:(b + 1) * TOK],
                kT_sb[p0:p0 + 64, m, b * N:(b + 1) * N],
                start=True, stop=True)
            # logits are small (|l| < ~4); exp without max-subtraction is safe
            attn16 = smp.tile([TOK, N], F16, tag="attn16", name="attn16")
            rowsum = smp.tile([TOK, 1], F32, tag="rowsum", name="rowsum")
            nc.scalar.activation(out=attn16, in_=aps, func=AF.Exp,
                                 scale=1.0, accum_out=rowsum)
            rinv = smp.tile([TOK, 1], F32, tag="rinv", name="rinv")
            nc.vector.reciprocal(out=rinv, in_=rowsum)
            nc.vector.tensor_scalar_mul(out=attn16, in0=attn16, scalar1=rinv)
            for jt in range(JT):
                for q in range(4):
                    nc.vector.transpose(
                        out=attnT[b][32 * q:32 * (q + 1), jt,
                                     h * TOK:(h + 1) * TOK],
                        in_=attn16[:, jt * 128 + 32 * q:
                                   jt * 128 + 32 * (q + 1)])

        def v_piece(b, jt, ch):
            c0 = 384 * ch
            vps = vpsp.tile([128, 384], F32, tag="vps", name="vps")
            for kt in range(CK):
                nc.tensor.matmul(
                    vps, xT_sb[:, kt, b * N + jt * 128:b * N + (jt + 1) * 128],
                    wv_sb[:, kt, c0:c0 + 384],
                    start=(kt == 0), stop=(kt == CK - 1))
            nc.scalar.copy(out=v_sb[b][:, jt, c0:c0 + 384], in_=vps)

        def d_token(b, il, dt, t):
            # accumulates onto the v-term already in ps_d; the last token's
            # final matmul closes the bank's accumulation group
            if b == 1 and il >= TOKA:
                ps, col, last = ps_dB, il - TOKA, il == TOK - 1
            else:
                ps, col = ps_d[b], il
                last = il == (TOKA - 1 if b == 1 else TOK - 1)
            for ck in range(CK):
                for half in range(2):
                    h = 2 * ck + half
                    for jt in range(JT):
                        nc.tensor.matmul(
                            ps[64 * half:64 * half + 64, ck, col:col + 1],
                            dt[:, t, jt, h * HD:(h + 1) * HD],
                            attnT[b][:, jt, h * TOK + il:h * TOK + il + 1],
                            start=False,
                            stop=(last and ck == CK - 1 and jt == JT - 1),
                            skip_group_check=True)

        def vterm_piece(b):
            # ck==0 jt==0 carries each bank's single start per partition
            # half; later writes zero-fill on first touch, then accumulate
            targets = ([(ps_d[0], 0, TOK)] if b == 0 else
                       [(ps_d[1], 0, TOKA), (ps_dB, TOKA, TOK)])
            for ps, t0, t1 in targets:
                for ck in range(CK):
                    for half in range(2):
                        h = 2 * ck + half
                        for jt in range(JT):
                            nc.tensor.matmul(
                                ps[64 * half:64 * half + 64, ck, :],
                                v_sb[b][:, jt, h * HD:(h + 1) * HD],
                                attnT[b][:, jt, h * TOK + t0:h * TOK + t1],
                                start=(ck == 0 and jt == 0), stop=False,
                                skip_group_check=True)

        def epi_piece(b):
            # normal-orientation projection for batch 0 / batch-1 tokens
            # 0:TOKA; all of it overlaps remaining d streaming
            nt = TOK if b == 0 else TOKA
            nc.vector.tensor_copy(out=hfinT[b][:, :, 0:nt],
                                  in_=ps_d[b])
            fps = fpsp.tile([TOK, C], F32, tag="fps", name="fps")
            # bias via ones-row matmul opens each 2KB zero region
            for lo, hi in ((0, 512), (512, 768)):
                nc.tensor.matmul(fps[0:nt, lo:hi], ones16[:, 0:nt],
                                 bias16[:, lo:hi],
                                 start=True, stop=False, skip_group_check=True)
            for ct in range(CK):
                for lo, hi in ((0, 512), (512, 768)):
                    nc.tensor.matmul(
                        fps[0:nt, lo:hi], hfinT[b][:, ct, 0:nt],
                        wp_sb[:, ct, lo:hi],
                        start=False, stop=(ct == CK - 1),
                        skip_group_check=True)
            # stage PSUM->SBUF split across DVE+ACT (halves the copy latency)
            nc.vector.tensor_copy(out=out_sb[b][0:nt, 0:384],
                                  in_=fps[0:nt, 0:384])
            nc.scalar.copy(out=out_sb[b][0:nt, 384:768],
                           in_=fps[0:nt, 384:768])
            # ACT-side HWDGE queue: doesn't block the SP queue's d streaming
            nc.scalar.dma_start(out=outp.ap()[b, 0:nt], in_=out_sb[b][0:nt])

        def epi_tail_piece():
            # batch-1 tail tokens, transposed projection: tiny free dims so
            # the post-last-chunk critical path is short
            hfB = hfinT[1][:, :, TOKA:TOK]
            nc.vector.tensor_copy(out=hfB, in_=ps_dB)
            oT = vpsp.tile([128, 384], F32, tag="vps", name="outTB")
            oTv = oT[:, 0:CK * TOKB].rearrange("p (ck i) -> p ck i", i=TOKB)
            for co in range(CK):
                nc.tensor.matmul(
                    oTv[:, co, :], bias16[:, 128 * co:128 * (co + 1)],
                    ones16[:, 0:TOKB],
                    start=(co == 0), stop=False, skip_group_check=True)
                for ct in range(CK):
                    nc.tensor.matmul(
                        oTv[:, co, :], wp_sb[:, ct, 128 * co:128 * (co + 1)],
                        hfB[:, ct, :],
                        start=False, stop=(co == CK - 1 and ct == CK - 1),
                        skip_group_check=True)
            oT_sb = singles.tile([128, CK, TOKB], F16, name="oT_sb")
            nc.vector.tensor_copy(out=oT_sb, in_=oTv)
            # SP queue is drained by now; its issue path is ~170ns shorter
            nc.sync.dma_start(out=outpT.ap(), in_=oT_sb)

        # ---- phase A: batch-0 attention + v, open batch-0 psum groups ----
        for m in range(CK):
            kq_piece(0, m)
            attn_piece(0, 2 * m)
            attn_piece(0, 2 * m + 1)
        for jt in range(JT):
            for ch in range(2):
                v_piece(0, jt, ch)
        vterm_piece(0)
        # ---- phase A.5: batch-1 attention + v (as d-loop side pieces) ----
        sides = [
            (1, lambda: kq_piece(1, 0)),
            (1, lambda: attn_piece(1, 0)), (1, lambda: attn_piece(1, 1)),
            (2, lambda: kq_piece(1, 1)),
            (2, lambda: attn_piece(1, 2)), (2, lambda: attn_piece(1, 3)),
            (3, lambda: kq_piece(1, 2)),
            (3, lambda: attn_piece(1, 4)), (3, lambda: attn_piece(1, 5)),
            (4, lambda: kq_piece(1, 3)),
            (4, lambda: attn_piece(1, 6)), (4, lambda: attn_piece(1, 7)),
            (5, lambda: kq_piece(1, 4)),
            (5, lambda: attn_piece(1, 8)), (5, lambda: attn_piece(1, 9)),
            (6, lambda: kq_piece(1, 5)),
            (6, lambda: attn_piece(1, 10)), (6, lambda: attn_piece(1, 11)),
            (7, lambda: v_piece(1, 0, 0)), (7, lambda: v_piece(1, 0, 1)),
            (7, lambda: v_piece(1, 1, 0)), (7, lambda: v_piece(1, 1, 1)),
            (8, lambda: vterm_piece(1)), (8, lambda: epi_piece(0)),
            # attention-phase pools are idle from here; draining them
            # mid-stream keeps their teardown out of the final cascade
            (10, lambda: apsp.release()),
            (10, lambda: kqps.release()),
            (10, lambda: smp.release()),
            (14, lambda: epi_piece(1)),
        ]
        if SERIAL:
            for m in range(CK):
                kq_piece(1, m)
                attn_piece(1, 2 * m)
                attn_piece(1, 2 * m + 1)
            for jt in range(JT):
                for ch in range(2):
                    v_piece(1, jt, ch)
            vterm_piece(1)
            sides = []
        emitted = 0
        chunks = [(b, ic0) for b in range(B) for ic0 in range(0, TOK, NTOK)]
        for ci, (b, ic0) in enumerate(chunks):
            while emitted < len(sides) and sides[emitted][0] <= ci:
                sides[emitted][1]()
                emitted += 1
            dt = dpool.tile([128, NTOK, JT, C], F8, name="d_tile")
            nc.sync.dma_start(
                out=dt,
                in_=dsl.ap()[b, ic0:ic0 + NTOK].rearrange(
                    "t (jt p) c -> p t jt c", p=128))
            for t in range(NTOK):
                d_token(b, ic0 + t, dt, t)
        while emitted < len(sides):
            sides[emitted][1]()
            emitted += 1

        # ---- tail: batch-1 tail-token epilogue ----
        if SERIAL:
            epi_piece(0)
            epi_piece(1)
        epi_tail_piece()

        if DEBUG_TAPS:
            d_kT = nc.dram_tensor("d_kT", [128, CK, B * N], F16,
                                  kind="ExternalOutput")
            d_qT = nc.dram_tensor("d_qT", [128, CK, B * TOK], F16,
                                  kind="ExternalOutput")
            d_attnT = nc.dram_tensor("d_attnT", [B, 128, JT, H * TOK], F16,
                                     kind="ExternalOutput")
            d_hfinT = nc.dram_tensor("d_hfinT", [B, 128, CK, TOK], F16,
                                     kind="ExternalOutput")
            d_v = nc.dram_tensor("d_v", [B, 128, JT, C], F16,
                                 kind="ExternalOutput")
            nc.sync.dma_start(out=d_kT.ap(), in_=kT_sb)
            nc.sync.dma_start(out=d_qT.ap(), in_=qT_sb)
            for b in range(B):
                nc.sync.dma_start(out=d_attnT.ap()[b], in_=attnT[b])
                nc.sync.dma_start(out=d_hfinT.ap()[b], in_=hfinT[b])
                nc.sync.dma_start(out=d_v.ap()[b], in_=v_sb[b])

        for p in reversed(stack):
            p.release()

    nc.compile()
    return nc


def make_in_maps(x, d, w_qkv, w_proj, b_proj):
    x = np.asarray(x, dtype=np.float32)
    w_qkv = np.asarray(w_qkv, dtype=np.float32)
    w_proj = np.asarray(w_proj, dtype=np.float32)
    b_proj = np.asarray(b_proj, dtype=np.float32)

    scale = HD ** -0.5
    wq = np.ascontiguousarray((w_qkv[0:C] * scale).T)
    wk = np.ascontiguousarray(w_qkv[C:2 * C].T)
    wkqT = np.concatenate([wq, wk], axis=1).astype(np.float16)   # [C, 2C]
    wvT = np.ascontiguousarray(w_qkv[2 * C:3 * C].T).astype(np.float16)
    wpT = np.ascontiguousarray(w_proj.T).astype(np.float16)
    xTf = np.ascontiguousarray(
        x.reshape(B * N, C).T).astype(np.float16)                # [C, B*N]
    d8 = np.asarray(d, dtype=np.float32).astype(ml_dtypes.float8_e3m4)

    in_maps = []
    for c in range(NCORES):
        i0 = c * TOK
        xq = x[:, i0:i0 + TOK, :].reshape(B * TOK, C)
        in_maps.append({
            "dsl": np.ascontiguousarray(d8[:, i0:i0 + TOK]),
            "wkqT": wkqT,
            "wvT": wvT,
            "wpT": wpT,
            "xT": xTf,
            "xqT": np.ascontiguousarray(xq.T).astype(np.float16),
            "bproj": b_proj.reshape(1, C).astype(np.float16),
        })
    return in_maps


def kernel(x, d, w_qkv, w_proj, b_proj):
    global _CACHED_NC
    if _CACHED_NC is None:
        _CACHED_NC = build_nc()
    nc = _CACHED_NC

    in_maps = make_in_maps(x, d, w_qkv, w_proj, b_proj)
    res = run_bass_kernel_spmd(nc, in_maps, core_ids=list(range(NCORES)))

    out = np.empty((B, N, C), dtype=np.float32)
    for c in range(NCORES):
        i0 = c * TOK
        out[:, i0:i0 + TOK, :] = \
            np.asarray(res.results[c]["outp"]).astype(np.float32)
        # batch-1 tail tokens come back transposed: [cc, ck, i]
        oT = np.asarray(res.results[c]["outpT"]).astype(np.float32)
        out[1, i0 + TOKA:i0 + TOK, :] = \
            oT.transpose(2, 1, 0).reshape(TOKB, C)
    return out


# revision 61
# speedup vs baseline: 1.0199x; 1.0012x over previous
"""Trainium2 Bass kernel for nn_Attention_D (pairwise-bias attention).

Problem: B=2, N=256, C=768, H=12, hd=64
  qkv = x @ w_qkv.T ; attn = softmax(q k^T * hd^-0.5)
  out = attn @ v + einsum('bhij,bhijd->bhid', attn, dh); out @ w_proj.T + b

d [B, N, N, C] dominates; the kernel is DMA-bound (global DMA pipe
~360 B/ns per core). Query rows are sharded across the 8 cores (32 per
batch per core); d streams in as float8_e3m4 (1 byte, ~1e-2 rel err vs
the 2e-2 gate; fp8 weights/attn tested and rejected), everything else
fp16. Per-core DMA ~51.4 us of the ~58.7 us total.

The d-term out2[h,i,c] = sum_j attn[h,i,j] * d[i,j,c] is computed entirely
on PE in transposed form: per token i, per 128-wide c-chunk ck and 64-wide
half (head h = 2*ck + half), a tiny matmul
    psum[c, i-col] += d_i[j, c-block].T(lhsT, e3m4) @ attnT[:, h-col](fp16)
accumulates the diagonal-block result directly into a [c, token] PSUM
layout (out free size 1 -> ~no PE time; PE reads e3m4 straight from the
DMA tile; mixed e3m4 x fp16 matmul validated on HW). The v-term
(v.T as lhsT, attnT as rhs) accumulates into the same PSUM region, so the
epilogue is a single PSUM->SBUF fp16 copy per (b, ck) producing hfinT in
exactly the lhsT layout the final projection needs. No DVE d-path, no
diagonal extraction, no transposes of the output.
"""

import numpy as np
import ml_dtypes

import concourse.bass as bass
import concourse.bacc as bacc
import concourse.mybir as mybir
import concourse.tile as tile
from concourse.bass_utils import run_bass_kernel_spmd

B, N, C = 2, 256, 768
H, HD = 12, 64
NCORES = 8
TOK = N // NCORES          # 32 own query rows per batch per core
CK = C // 128              # 6 c-chunks
JT = N // 128              # 2 j partition tiles
NTOK = 4                   # tokens per d DMA chunk
TOKA = 24                  # batch-1 tokens projected early (rest: tail path)
TOKB = TOK - TOKA
F32 = mybir.dt.float32
F16 = mybir.dt.float16
F8 = mybir.dt.float8e3     # e3m4
AF = mybir.ActivationFunctionType

_CACHED_NC = None
DEBUG_TAPS = False
SERIAL = False


def build_nc():
    nc = bacc.Bacc("TRN2", target_bir_lowering=False, debug=False,
                   num_devices=NCORES)

    dsl = nc.dram_tensor("dsl", [B, TOK, N, C], F8, kind="ExternalInput")
    # cols 0:C = w_q.T * hd^-0.5, C:2C = w_k.T
    wkqT = nc.dram_tensor("wkqT", [C, 2 * C], F16, kind="ExternalInput")
    wvT = nc.dram_tensor("wvT", [C, C], F16, kind="ExternalInput")
    wpT = nc.dram_tensor("wpT", [C, C], F16, kind="ExternalInput")
    xT = nc.dram_tensor("xT", [C, B * N], F16, kind="ExternalInput")
    xqT = nc.dram_tensor("xqT", [C, B * TOK], F16, kind="ExternalInput")
    bproj = nc.dram_tensor("bproj", [1, C], F16, kind="ExternalInput")
    outp = nc.dram_tensor("outp", [B, TOK, C], F16, kind="ExternalOutput")
    # batch-1 tail tokens, transposed: [cc, ck, i] -> out[1, TOKA+i, 128*ck+cc]
    outpT = nc.dram_tensor("outpT", [128, CK, TOKB], F16,
                           kind="ExternalOutput")

    with tile.TileContext(nc) as tc:
        singles = tc.alloc_tile_pool(name="singles", bufs=1)
        dpool = tc.alloc_tile_pool(name="dpool", bufs=6)
        pss0 = tc.alloc_tile_pool(name="pss0", bufs=1, space="PSUM")
        pss1 = tc.alloc_tile_pool(name="pss1", bufs=1, space="PSUM")
        pssB = tc.alloc_tile_pool(name="pssB", bufs=1, space="PSUM")
        vpsp = tc.alloc_tile_pool(name="vpsp", bufs=1, space="PSUM")
        fpsp = tc.alloc_tile_pool(name="fpsp", bufs=1, space="PSUM")
        # attention-phase pools go on top of the pool stack: they are
        # released mid-stream (LIFO order) once batch-1 attention is done
        smp = tc.alloc_tile_pool(name="smp", bufs=3)
        kqps = tc.alloc_tile_pool(name="kqps", bufs=1, space="PSUM")
        apsp = tc.alloc_tile_pool(name="apsp", bufs=1, space="PSUM")
        stack = [singles, dpool, pss0, pss1, pssB, vpsp, fpsp]

        # ---- SBUF tiles (all fit; no pool cycling needed) ----
        wkq_sb = singles.tile([128, CK, 2 * C], F16, name="wkq_sb")
        wv_sb = singles.tile([128, CK, C], F16, name="wv_sb")
        wp_sb = singles.tile([128, CK, C], F16, name="wp_sb")
        xT_sb = singles.tile([128, CK, B * N], F16, name="xT_sb")
        xqT_sb = singles.tile([128, CK, B * TOK], F16, name="xqT_sb")
        kT_sb = singles.tile([128, CK, B * N], F16, name="kT_sb")
        qT_sb = singles.tile([128, CK, B * TOK], F16, name="qT_sb")
        v_sb = [singles.tile([128, JT, C], F16, name=f"v{b}") for b in range(B)]
        attnT = [singles.tile([128, JT, H * TOK], F16, name=f"attnT{b}")
                 for b in range(B)]
        hfinT = [singles.tile([128, CK, TOK], F16, name=f"hfinT{b}")
                 for b in range(B)]
        bias16 = singles.tile([1, C], F16, name="bias16")
        ones16 = singles.tile([1, TOK], F16, name="ones16")
        nc.gpsimd.memset(ones16, 1.0)
        out_sb = [singles.tile([TOK, C], F16, name=f"out_sb{b}")
                  for b in range(B)]

        # long-lived PSUM accumulators: d-term + v-term, [c, token] layout.
        # In separate banks: start_tensor_calc marks the whole 2KB zero
        # region pending-zero, so each bank gets exactly one start (per
        # partition half) and one stop. Batch 1 is split into an early part
        # (tokens 0:TOKA) and a small tail part so the final epilogue after
        # the last d chunk is cheap.
        ps_d = [pss0.tile([128, CK, TOK], F32, name="ps_d0"),
                pss1.tile([128, CK, TOKA], F32, name="ps_d1A")]
        ps_dB = pssB.tile([128, CK, TOKB], F32, name="ps_d1B")

        # ---- input DMAs, in intended DMA-engine FIFO order ----
        def load_w_cols(dst, src, c0, c1):
            nc.sync.dma_start(
                out=dst[:, :, c0:c1],
                in_=src.ap()[:, c0:c1].rearrange("(ko ki) co -> ki ko co",
                                                 ki=128))

        # k01, q01 first (gate the first kq piece), then x, then the rest
        load_w_cols(wkq_sb, wkqT, C, C + 256)
        load_w_cols(wkq_sb, wkqT, 0, 256)
        nc.sync.dma_start(
            out=xT_sb[:, :, 0:N],
            in_=xT.ap()[:, 0:N].rearrange("(ko ki) t -> ki ko t", ki=128))
        nc.sync.dma_start(
            out=xqT_sb, in_=xqT.ap().rearrange("(ko ki) t -> ki ko t", ki=128))
        load_w_cols(wkq_sb, wkqT, C + 256, C + 512)
        load_w_cols(wkq_sb, wkqT, 256, 512)
        load_w_cols(wkq_sb, wkqT, C + 512, C + 768)
        load_w_cols(wkq_sb, wkqT, 512, 768)
        nc.sync.dma_start(
            out=wv_sb, in_=wvT.ap().rearrange("(ko ki) co -> ki ko co", ki=128))
        nc.sync.dma_start(
            out=xT_sb[:, :, N:2 * N],
            in_=xT.ap()[:, N:2 * N].rearrange("(ko ki) t -> ki ko t", ki=128))
        nc.sync.dma_start(out=bias16, in_=bproj.ap())
        nc.sync.dma_start(
            out=wp_sb, in_=wpT.ap().rearrange("(ko ki) co -> ki ko co", ki=128))

        # ---- emission helpers ----
        def kq_piece(b, m):
            kps = kqps.tile([128, N + TOK], F32, tag="kqp", name="kqp")
            for kt in range(CK):
                nc.tensor.matmul(
                    kps[:, 0:N], wkq_sb[:, kt, C + m * 128:C + (m + 1) * 128],
                    xT_sb[:, kt, b * N:(b + 1) * N],
                    start=(kt == 0), stop=(kt == CK - 1))
            nc.scalar.copy(out=kT_sb[:, m, b * N:(b + 1) * N], in_=kps[:, 0:N])
            for kt in range(CK):
                nc.tensor.matmul(
                    kps[:, N:N + TOK], wkq_sb[:, kt, m * 128:(m + 1) * 128],
                    xqT_sb[:, kt, b * TOK:(b + 1) * TOK],
                    start=(kt == 0), stop=(kt == CK - 1))
            nc.scalar.copy(out=qT_sb[:, m, b * TOK:(b + 1) * TOK],
                           in_=kps[:, N:N + TOK])

        def attn_piece(b, h):
            p0 = 64 * (h % 2)
            m = h // 2
            aps = apsp.tile([TOK, N], F32, tag="aps", name="aps")
            nc.tensor.matmul(
                aps, qT_sb[p0:p0 + 64, m, b * TOK:(b + 1) * TOK],
                kT_sb[p0:p0 + 64, m, b * N:(b + 1) * N],
                start=True, stop=True)
            # logits are small (|l| < ~4); exp without max-subtraction is safe
            attn16 = smp.tile([TOK, N], F16, tag="attn16", name="attn16")
            rowsum = smp.tile([TOK, 1], F32, tag="rowsum", name="rowsum")
            nc.scalar.activation(out=attn16, in_=aps, func=AF.Exp,
                                 scale=1.0, accum_out=rowsum)
            rinv = smp.tile([TOK, 1], F32, tag="rinv", name="rinv")
            nc.vector.reciprocal(out=rinv, in_=rowsum)
            nc.vector.tensor_scalar_mul(out=attn16, in0=attn16, scalar1=rinv)
            for jt in range(JT):
                for q in range(4):
                    nc.vector.transpose(
                        out=attnT[b][32 * q:32 * (q + 1), jt,
                                     h * TOK:(h + 1) * TOK],
                        in_=attn16[:, jt * 128 + 32 * q:
                                   jt * 128 + 32 * (q + 1)])

        def v_piece(b, jt, ch):
            c0 = 384 * ch
            vps = vpsp.tile([128, 384], F32, tag="vps", name="vps")
            for kt in range(CK):
                nc.tensor.matmul(
                    vps, xT_sb[:, kt, b * N + jt * 128:b * N + (jt + 1) * 128],
                    wv_sb[:, kt, c0:c0 + 384],
                    start=(kt == 0), stop=(kt == CK - 1))
            nc.scalar.copy(out=v_sb[b][:, jt, c0:c0 + 384], in_=vps)

        def d_token(b, il, dt, t):
            # accumulates onto the v-term already in ps_d; the last token's
            # final matmul closes the bank's accumulation group
            if b == 1 and il >= TOKA:
                ps, col, last = ps_dB, il - TOKA, il == TOK - 1
            else:
                ps, col = ps_d[b], il
                last = il == (TOKA - 1 if b == 1 else TOK - 1)
            for ck in range(CK):
                for half in range(2):
                    h = 2 * ck + half
                    for jt in range(JT):
                        nc.tensor.matmul(
                            ps[64 * half:64 * half + 64, ck, col:col + 1],
                            dt[:, t, jt, h * HD:(h + 1) * HD],
                            attnT[b][:, jt, h * TOK + il:h * TOK + il + 1],
                            start=False,
                            stop=(last and ck == CK - 1 and jt == JT - 1),
                            skip_group_check=True)

        def vterm_piece(b):
            # ck==0 jt==0 carries each bank's single start per partition
            # half; later writes zero-fill on first touch, then accumulate
            targets = ([(ps_d[0], 0, TOK)] if b == 0 else
                       [(ps_d[1], 0, TOKA), (ps_dB, TOKA, TOK)])
            for ps, t0, t1 in targets:
                for ck in range(CK):
                    for half in range(2):
                        h = 2 * ck + half
                        for jt in range(JT):
                            nc.tensor.matmul(
                                ps[64 * half:64 * half + 64, ck, :],
                                v_sb[b][:, jt, h * HD:(h + 1) * HD],
                                attnT[b][:, jt, h * TOK + t0:h * TOK + t1],
                                start=(ck == 0 and jt == 0), stop=False,
                                skip_group_check=True)

        def epi_piece(b):
            # normal-orientation projection for batch 0 / batch-1 tokens
            # 0:TOKA; all of it overlaps remaining d streaming
            nt = TOK if b == 0 else TOKA
            nc.vector.tensor_copy(out=hfinT[b][:, :, 0:nt],
                                  in_=ps_d[b])
            fps = fpsp.tile([TOK, C], F32, tag="fps", name="fps")
            # bias via ones-row matmul opens each 2KB zero region
            for lo, hi in ((0, 512), (512, 768)):
                nc.tensor.matmul(fps[0:nt, lo:hi], ones16[:, 0:nt],
                                 bias16[:, lo:hi],
                                 start=True, stop=False, skip_group_check=True)
            for ct in range(CK):
                for lo, hi in ((0, 512), (512, 768)):
                    nc.tensor.matmul(
                        fps[0:nt, lo:hi], hfinT[b][:, ct, 0:nt],
                        wp_sb[:, ct, lo:hi],
                        start=False, stop=(ct == CK - 1),
                        skip_group_check=True)
            # stage PSUM->SBUF split across DVE+ACT (halves the copy latency)
            nc.vector.tensor_copy(out=out_sb[b][0:nt, 0:384],
                                  in_=fps[0:nt, 0:384])
            nc.scalar.copy(out=out_sb[b][0:nt, 384:768],
                           in_=fps[0:nt, 384:768])
            # ACT-side HWDGE queue: doesn't block the SP queue's d streaming
            nc.scalar.dma_start(out=outp.ap()[b, 0:nt], in_=out_sb[b][0:nt])

        # batch-1 tail tokens, transposed projection with tiny free dims.
        # Tokens TOKA:TOKA+4 are final after the second-to-last chunk, so
        # their half of the copy/proj/stage chain runs hidden; only the last
        # chunk's 4 columns remain on the post-stream critical path.
        oT_sb = singles.tile([128, CK, TOKB], F16, name="oT_sb")
        tail_st = {}

        def epi_tail_start():
            hfB = hfinT[1][:, :, TOKA:TOK]
            nc.vector.tensor_copy(out=hfB[:, :, 0:4], in_=ps_dB[:, :, 0:4])
            oT = vpsp.tile([128, 384], F32, tag="vps", name="outTB")
            oTv = oT[:, 0:CK * TOKB].rearrange("p (ck i) -> p ck i", i=TOKB)
            tail_st["oTv"] = oTv
            for co in range(CK):
                # full-width bias opens the bank's zero region (no data dep)
                nc.tensor.matmul(
                    oTv[:, co, :], bias16[:, 128 * co:128 * (co + 1)],
                    ones16[:, 0:TOKB],
                    start=(co == 0), stop=False, skip_group_check=True)
                for ct in range(CK):
                    nc.tensor.matmul(
                        oTv[:, co, 0:4],
                        wp_sb[:, ct, 128 * co:128 * (co + 1)],
                        hfB[:, ct, 0:4],
                        start=False, stop=False, skip_group_check=True)
            nc.vector.tensor_copy(out=oT_sb[:, :, 0:4], in_=oTv[:, :, 0:4])

        def epi_tail_end():
            hfB = hfinT[1][:, :, TOKA:TOK]
            oTv = tail_st["oTv"]
            nc.vector.tensor_copy(out=hfB[:, :, 4:8], in_=ps_dB[:, :, 4:8])
            for co in range(CK):
                for ct in range(CK):
                    nc.tensor.matmul(
                        oTv[:, co, 4:8],
                        wp_sb[:, ct, 128 * co:128 * (co + 1)],
                        hfB[:, ct, 4:8],
                        start=False, stop=(co == CK - 1 and ct == CK - 1),
                        skip_group_check=True)
            nc.vector.tensor_copy(out=oT_sb[:, :, 4:8], in_=oTv[:, :, 4:8])
            # SP queue is drained by now; its issue path is ~170ns shorter
            nc.sync.dma_start(out=outpT.ap(), in_=oT_sb)

        # ---- phase A: batch-0 attention + v, open batch-0 psum groups ----
        for m in range(CK):
            kq_piece(0, m)
            attn_piece(0, 2 * m)
            attn_piece(0, 2 * m + 1)
        for jt in range(JT):
            for ch in range(2):
                v_piece(0, jt, ch)
        vterm_piece(0)
        # ---- phase A.5: batch-1 attention + v (as d-loop side pieces) ----
        sides = [
            (1, lambda: kq_piece(1, 0)),
            (1, lambda: attn_piece(1, 0)), (1, lambda: attn_piece(1, 1)),
            (2, lambda: kq_piece(1, 1)),
            (2, lambda: attn_piece(1, 2)), (2, lambda: attn_piece(1, 3)),
            (3, lambda: kq_piece(1, 2)),
            (3, lambda: attn_piece(1, 4)), (3, lambda: attn_piece(1, 5)),
            (4, lambda: kq_piece(1, 3)),
            (4, lambda: attn_piece(1, 6)), (4, lambda: attn_piece(1, 7)),
            (5, lambda: kq_piece(1, 4)),
            (5, lambda: attn_piece(1, 8)), (5, lambda: attn_piece(1, 9)),
            (6, lambda: kq_piece(1, 5)),
            (6, lambda: attn_piece(1, 10)), (6, lambda: attn_piece(1, 11)),
            (7, lambda: v_piece(1, 0, 0)), (7, lambda: v_piece(1, 0, 1)),
            (7, lambda: v_piece(1, 1, 0)), (7, lambda: v_piece(1, 1, 1)),
            (8, lambda: vterm_piece(1)), (8, lambda: epi_piece(0)),
            # attention-phase pools are idle from here; draining them
            # mid-stream keeps their teardown out of the final cascade
            (10, lambda: apsp.release()),
            (10, lambda: kqps.release()),
            (10, lambda: smp.release()),
            (14, lambda: epi_piece(1)),
            (15, lambda: epi_tail_start()),
        ]
        if SERIAL:
            for m in range(CK):
                kq_piece(1, m)
                attn_piece(1, 2 * m)
                attn_piece(1, 2 * m + 1)
            for jt in range(JT):
                for ch in range(2):
                    v_piece(1, jt, ch)
            vterm_piece(1)
            sides = []
        emitted = 0
        chunks = [(b, ic0) for b in range(B) for ic0 in range(0, TOK, NTOK)]
        for ci, (b, ic0) in enumerate(chunks):
            while emitted < len(sides) and sides[emitted][0] <= ci:
                sides[emitted][1]()
                emitted += 1
            dt = dpool.tile([128, NTOK, JT, C], F8, name="d_tile")
            nc.sync.dma_start(
                out=dt,
                in_=dsl.ap()[b, ic0:ic0 + NTOK].rearrange(
                    "t (jt p) c -> p t jt c", p=128))
            for t in range(NTOK):
                d_token(b, ic0 + t, dt, t)
        while emitted < len(sides):
            sides[emitted][1]()
            emitted += 1

        # ---- tail: batch-1 tail-token epilogue ----
        if SERIAL:
            epi_piece(0)
            epi_piece(1)
            epi_tail_start()
        epi_tail_end()

        if DEBUG_TAPS:
            d_kT = nc.dram_tensor("d_kT", [128, CK, B * N], F16,
                                  kind="ExternalOutput")
            d_qT = nc.dram_tensor("d_qT", [128, CK, B * TOK], F16,
                                  kind="ExternalOutput")
            d_attnT = nc.dram_tensor("d_attnT", [B, 128, JT, H * TOK], F16,
                                     kind="ExternalOutput")
            d_hfinT = nc.dram_tensor("d_hfinT", [B, 128, CK, TOK], F16,
                                     kind="ExternalOutput")
            d_v = nc.dram_tensor("d_v", [B, 128, JT, C], F16,
                                 kind="ExternalOutput")
            nc.sync.dma_start(out=d_kT.ap(), in_=kT_sb)
            nc.sync.dma_start(out=d_qT.ap(), in_=qT_sb)
            for b in range(B):
                nc.sync.dma_start(out=d_attnT.ap()[b], in_=attnT[b])
                nc.sync.dma_start(out=d_hfinT.ap()[b], in_=hfinT[b])
                nc.sync.dma_start(out=d_v.ap()[b], in_=v_sb[b])

        for p in reversed(stack):
            p.release()

    nc.compile()
    return nc


def make_in_maps(x, d, w_qkv, w_proj, b_proj):
    x = np.asarray(x, dtype=np.float32)
    w_qkv = np.asarray(w_qkv, dtype=np.float32)
    w_proj = np.asarray(w_proj, dtype=np.float32)
    b_proj = np.asarray(b_proj, dtype=np.float32)

    scale = HD ** -0.5
    wq = np.ascontiguousarray((w_qkv[0:C] * scale).T)
    wk = np.ascontiguousarray(w_qkv[C:2 * C].T)
    wkqT = np.concatenate([wq, wk], axis=1).astype(np.float16)   # [C, 2C]
    wvT = np.ascontiguousarray(w_qkv[2 * C:3 * C].T).astype(np.float16)
    wpT = np.ascontiguousarray(w_proj.T).astype(np.float16)
    xTf = np.ascontiguousarray(
        x.reshape(B * N, C).T).astype(np.float16)                # [C, B*N]
    d8 = np.asarray(d, dtype=np.float32).astype(ml_dtypes.float8_e3m4)

    in_maps = []
    for c in range(NCORES):
        i0 = c * TOK
        xq = x[:, i0:i0 + TOK, :].reshape(B * TOK, C)
        in_maps.append({
            "dsl": np.ascontiguousarray(d8[:, i0:i0 + TOK]),
            "wkqT": wkqT,
            "wvT": wvT,
            "wpT": wpT,
            "xT": xTf,
            "xqT": np.ascontiguousarray(xq.T).astype(np.float16),
            "bproj": b_proj.reshape(1, C).astype(np.float16),
        })
    return in_maps


def kernel(x, d, w_qkv, w_proj, b_proj):
    global _CACHED_NC
    if _CACHED_NC is None:
        _CACHED_NC = build_nc()
    nc = _CACHED_NC

    in_maps = make_in_maps(x, d, w_qkv, w_proj, b_proj)
    res = run_bass_kernel_spmd(nc, in_maps, core_ids=list(range(NCORES)))

    out = np.empty((B, N, C), dtype=np.float32)
    for c in range(NCORES):
        i0 = c * TOK
        out[:, i0:i0 + TOK, :] = \
            np.asarray(res.results[c]["outp"]).astype(np.float32)
        # batch-1 tail tokens come back transposed: [cc, ck, i]
        oT = np.asarray(res.results[c]["outpT"]).astype(np.float32)
        out[1, i0 + TOKA:i0 + TOK, :] = \
            oT.transpose(2, 1, 0).reshape(TOKB, C)
    return out
